# revision 7
# baseline (speedup 1.0000x reference)
"""Energy refinement kernel for Trainium2 (8 NeuronCores, SPMD row-sharded).

Math notes
----------
reference() computes, for L=4096 coords [L,3] and a 0/1 contact_map [L,L]:
  e_bond  = mean((||c[i+1]-c[i]|| - 6)^2)                       (O(L), host)
  d[i,j]  = ||c_i - c_j|| (+1e-8)
  e_clash = sum_{j>=i+3} relu(3.4-d)^2 / L
  e_pair  = sum_{contact & |i-j|>=3} (d-9)^2 / max(n_contacts,1)
  total   = e_bond + 2*e_clash + 0.5*e_pair

Device strategy (folded symmetry, row-sharded over 8 cores):
  d2 = A @ B^T in ONE fp16 matmul with a hi/lo split (K=13):
    c = hi + lo (both fp16), |c|^2 = n2hi + n2lo (both fp16); the K slots
    pair up so sum_k A[i,k]B[j,k] = |ci|^2+|cj|^2-2ci.cj up to ~0.01 abs
    (the dropped lo*lo cross term is ~1e-2); fp16 streams the PE at 1
    cycle/row vs fp32's 4.
  d  = sqrt(d2 + EPS)  (ACT pass straight from PSUM; the +EPS bias both
  clamps fp32/fp16 round-off away from negative and replaces the
  baseline's separate relu pass; output fp16)
  d is symmetric, so each unordered pair is visited once: a 128-row block a
  only processes the 15-block cyclic column span [128(a+1), 128a+2048) —
  block offsets 1..15 of 32.  Per 128-row tile the device accumulates
  per-partition sums of:
    clash = (min(d,3.4)-3.4)^2   via DVE min/sub (4x fp16 mode) then a
            column-split square+reduce: first CLASH_ACT_COLS on ACT
            (Square w/ accum), rest on DVE tensor_tensor_reduce
    pair  = w*(d-9)^2, w = (c_ij+c_ji) masked by |i-j|>=3 on host
            (values {0,1,2}, exact in fp16), via ACT Square(d-9) then
            DVE tensor_tensor_reduce(p, w)
  The within-block (offset-0) and offset-16 pairs — the blocks that would
  be double-counted — are ~0.5M pairs and are computed EXACTLY on the host
  in float64 instead.  Per-core column spans and w ship as data
  (pre-gathered B columns), keeping the SPMD program identical on all cores.
Host finishing (float64): add the exact diag/sep-16 block terms, subtract
the block-crossing |i-j|<=2 band CLASH pairs (emulated with the device's
own formula so they cancel; pair needs no band fix — w is pre-masked),
divide, add the bond term.
"""

import numpy as np

L = 4096
NCORES = 8
RPC = L // NCORES          # 512 rows per core
RT = RPC // 128            # 4 row tiles of 128 partitions
SPAN = 15 * 128            # 1920 columns per row tile (block offsets 1..15)
KS = 13                    # fp16-split matmul contraction depth
MIN_DIST = 3.4
TARGET_DIST = 9.0
IDEAL_BOND = 6.0
W_BOND, W_CLASH, W_PAIR = 1.0, 2.0, 0.5
EPS = 0.05                 # d2 clamp bias inside the sqrt activation
CLASH_ACT_COLS = 512       # clash square+reduce: [0,s) on ACT, [s,SPAN) on DVE


def _build_nc(reps=1):
    import concourse.bass as bass
    import concourse.bacc as bacc
    import concourse.mybir as mybir
    import concourse.tile as tile

    f32 = mybir.dt.float32
    f16 = mybir.dt.float16
    AF = mybir.ActivationFunctionType
    ALU = mybir.AluOpType

    # Bacc (not Bass): its compile() runs move_matmul_waits_to_ldweights,
    # required because walrus allows only one sync wait per Matmult.
    nc = bacc.Bacc(None)
    # ab = [at | bt-span x4] so a single DMA (one wait semaphore) loads all
    # matmul operands — walrus allows only one sync wait per Matmult.
    ab = nc.declare_dram_parameter("ab", [KS, RPC + RT * SPAN], f16, isOutput=False)
    wmap = nc.declare_dram_parameter("wmap", [RPC, SPAN], f16, isOutput=False)
    o_clash = nc.declare_dram_parameter("o_clash", [128, 2 * RT], f32, isOutput=True)
    o_pair = nc.declare_dram_parameter("o_pair", [128, RT], f32, isOutput=True)

    SA = CLASH_ACT_COLS

    with tile.TileContext(nc) as tc:
        with (
            tc.tile_pool(name="const", bufs=1) as constp,
            tc.tile_pool(name="wp", bufs=3) as wp,
            tc.tile_pool(name="work", bufs=2) as work,
            tc.tile_pool(name="accp", bufs=1) as accp,
            tc.tile_pool(name="psum", bufs=2, space=bass.MemorySpace.PSUM) as psum,
        ):
            ab_sb = constp.tile([KS, RPC + RT * SPAN], f16)
            bias_eps = constp.tile([128, 1], f32)
            nc.gpsimd.memset(bias_eps[:], EPS)
            bias_t = constp.tile([128, 1], f32)
            nc.gpsimd.memset(bias_t[:], -TARGET_DIST)
            # split the operand load so row-tile 0's matmuls start after the
            # first chunk instead of waiting for the whole transfer
            nc.sync.dma_start(ab_sb[:, : RPC + SPAN], ab[:, : RPC + SPAN])
            for it in range(1, RT):
                lo = RPC + it * SPAN
                nc.sync.dma_start(ab_sb[:, lo : lo + SPAN], ab[:, lo : lo + SPAN])

            acc_clash = accp.tile([128, 2 * RT], f32)
            acc_pair = accp.tile([128, RT], f32)

            for rep in range(reps):
                for it in range(RT):
                    wt = wp.tile([128, SPAN], f16, tag="wt")
                    nc.sync.dma_start(wt[:], wmap[it * 128 : (it + 1) * 128, :])
                    lhs = ab_sb[:, it * 128 : (it + 1) * 128]
                    rbase = RPC + it * SPAN
                    ps = psum.tile([128, SPAN], f32, tag="d2")
                    off = 0
                    for n in (512, 512, 512, 384):
                        nc.tensor.matmul(
                            ps[:, off : off + n],
                            lhs,
                            ab_sb[:, rbase + off : rbase + off + n],
                            start=True,
                            stop=True,
                        )
                        off += n

                    # d = sqrt(d2 + EPS): EPS clamps matmul round-off, so no
                    # separate relu pass; fp16 output feeds the DVE 4x mode
                    t_d = work.tile([128, SPAN], f16, tag="t_d")
                    nc.scalar.activation(t_d[:], ps[:], AF.Sqrt, bias=bias_eps[:])

                    # clash: m = min(d,3.4)-3.4 (DVE 4x), then m^2 summed —
                    # first SA cols on ACT (Square+accum), rest on DVE ttr
                    t_m = work.tile([128, SPAN], f16, tag="t_m")
                    nc.vector.tensor_scalar(
                        t_m[:], t_d[:], MIN_DIST, MIN_DIST, ALU.min, ALU.subtract
                    )
                    t_j0 = work.tile([128, SA], f16, tag="junk_a")
                    nc.scalar.activation(
                        t_j0[:],
                        t_m[:, :SA],
                        AF.Square,
                        accum_out=acc_clash[:, 2 * it : 2 * it + 1],
                    )
                    t_j1 = work.tile([128, SPAN - SA], f16, tag="junk_d")
                    nc.vector.scalar_tensor_tensor(
                        t_j1[:],
                        t_m[:, SA:],
                        1.0,
                        t_m[:, SA:],
                        ALU.mult,
                        ALU.mult,
                        accum_out=acc_clash[:, 2 * it + 1 : 2 * it + 2],
                    )

                    # pair: p = (d-9)^2 on ACT, then sum p*w on DVE ttr
                    t_p = work.tile([128, SPAN], f16, tag="t_p")
                    nc.scalar.activation(
                        t_p[:], t_d[:], AF.Square, bias=bias_t[:]
                    )
                    t_j2 = work.tile([128, SPAN], f16, tag="junk_p")
                    nc.vector.scalar_tensor_tensor(
                        t_j2[:],
                        t_p[:],
                        1.0,
                        wt[:],
                        ALU.mult,
                        ALU.mult,
                        accum_out=acc_pair[:, it : it + 1],
                    )

            nc.sync.dma_start(o_clash[:], acc_clash[:])
            nc.sync.dma_start(o_pair[:], acc_pair[:])
    nc.compile()
    return nc


def _split16(x):
    """x (f64/f32) -> (hi, lo) fp16 with hi + lo ~= x."""
    hi = x.astype(np.float16)
    lo = (x.astype(np.float64) - hi.astype(np.float64)).astype(np.float16)
    return hi, lo


def _augmented(coords):
    """AT, BT fp16 [13, L] such that sum_k AT[k,i]*BT[k,j] ~= ||c_i - c_j||^2."""
    c = np.asarray(coords, dtype=np.float64)
    n2 = (c * c).sum(axis=1)
    chi, clo = _split16(c)          # [L,3] each
    n2hi, n2lo = _split16(n2)       # [L]
    one = np.ones(c.shape[0], dtype=np.float16)
    zero = np.zeros(c.shape[0], dtype=np.float16)
    # K slots: 0-2 hi*-2hi, 3-5 hi*-2lo, 6-8 lo*-2hi, 9/10 n2hi/n2lo * 1,
    # 11/12 1 * n2hi/n2lo
    AT = np.stack(
        [chi[:, 0], chi[:, 1], chi[:, 2],
         chi[:, 0], chi[:, 1], chi[:, 2],
         clo[:, 0], clo[:, 1], clo[:, 2],
         n2hi, n2lo, one, one]
    ).astype(np.float16)
    m2hi = (-2.0 * chi.astype(np.float64)).astype(np.float16)
    m2lo = (-2.0 * clo.astype(np.float64)).astype(np.float16)
    BT = np.stack(
        [m2hi[:, 0], m2hi[:, 1], m2hi[:, 2],
         m2lo[:, 0], m2lo[:, 1], m2lo[:, 2],
         m2hi[:, 0], m2hi[:, 1], m2hi[:, 2],
         one, one, n2hi, n2lo]
    ).astype(np.float16)
    _ = zero
    return AT, BT


def _host_inputs(coords, contact_map):
    AT, BT = _augmented(coords)
    idx = np.arange(L)
    in_maps = []
    for r in range(NCORES):
        parts = [AT[:, r * RPC : (r + 1) * RPC]]
        w_r = np.empty((RPC, SPAN), dtype=np.float16)
        for it in range(RT):
            a = r * RT + it
            i0 = a * 128
            cols = np.arange(i0 + 128, i0 + 128 + SPAN) % L
            parts.append(BT[:, cols])
            w = (
                contact_map[i0 : i0 + 128][:, cols]
                + contact_map[cols][:, i0 : i0 + 128].T
            )
            # reference pair mask: |i-j| >= 3 on absolute indices
            seps = np.abs(cols[None, :] - (i0 + idx[:128])[:, None])
            w[seps < 3] = 0.0
            w_r[it * 128 : (it + 1) * 128] = w.astype(np.float16)
        in_maps.append(
            {
                "ab": np.ascontiguousarray(
                    np.concatenate(parts, axis=1), dtype=np.float16
                ),
                "wmap": w_r,
            }
        )
    return AT, BT, in_maps


def _host_block_terms(coords, contact_map):
    """Exact f64 clash/pair sums over the diag and sep-16 block pairs
    (the unordered pairs the device span skips), reference masks applied."""
    c = coords.astype(np.float64)
    clash_sum = 0.0
    pair_sum = 0.0
    for a in range(L // 128):
        i0 = a * 128
        # within-block pairs i<j
        blk = c[i0 : i0 + 128]
        dd = np.sqrt(((blk[:, None, :] - blk[None, :, :]) ** 2).sum(-1)) + 1e-8
        iu, ju = np.triu_indices(128, k=1)
        sep = ju - iu
        d_u = dd[iu, ju]
        cm = contact_map[i0 : i0 + 128][:, i0 : i0 + 128]
        cw = cm[iu, ju].astype(np.float64) + cm[ju, iu].astype(np.float64)
        m3 = sep >= 3
        cl = np.maximum(MIN_DIST - d_u[m3], 0.0)
        clash_sum += float((cl * cl).sum())
        pair_sum += float((cw[m3] * (d_u[m3] - TARGET_DIST) ** 2).sum())
        # sep-16 block pairs, visited once for a in [0, 16)
        if a < 16:
            j0 = i0 + 2048
            blk2 = c[j0 : j0 + 128]
            d2 = np.sqrt(
                ((blk[:, None, :] - blk2[None, :, :]) ** 2).sum(-1)
            ) + 1e-8
            cl2 = np.maximum(MIN_DIST - d2, 0.0)
            clash_sum += float((cl2 * cl2).sum())
            cw2 = contact_map[i0 : i0 + 128][:, j0 : j0 + 128].astype(
                np.float64
            ) + contact_map[j0 : j0 + 128][:, i0 : i0 + 128].T.astype(np.float64)
            pair_sum += float((cw2 * (d2 - TARGET_DIST) ** 2).sum())
    return clash_sum, pair_sum


def _band_crossing_clash(AT, BT):
    """Device-formula clash sums over block-CROSSING |i-j|<=2 pairs
    (the only banded pairs inside the device span), to subtract.
    Pair needs no correction: wmap is pre-masked by |i-j|>=3."""
    band_clash = 0.0
    for s_off in (1, 2):
        i = np.arange(L - s_off)  # non-wrapping pairs only
        i = i[(i % 128) >= 128 - s_off]  # block-crossing only
        j = i + s_off
        s = np.zeros(len(i), dtype=np.float64)
        for m in range(KS):
            s += AT[m, i].astype(np.float64) * BT[m, j].astype(np.float64)
        dh = np.sqrt(np.maximum(s, 0.0) + EPS).astype(np.float16).astype(np.float64)
        cl = np.minimum(dh, MIN_DIST) - MIN_DIST
        band_clash += float((cl * cl).sum())
    return band_clash


_CACHE = {}


def kernel(coords, contact_map):
    from concourse.bass_utils import run_bass_kernel_spmd

    coords = np.asarray(coords, dtype=np.float32)
    # reference semantics: a pair is a contact iff contact_map > 0.5
    contact_map = np.ascontiguousarray(
        (np.asarray(contact_map) > 0.5).astype(np.float32)
    )
    AT, BT, in_maps = _host_inputs(coords, contact_map)

    if "nc" not in _CACHE:
        _CACHE["nc"] = _build_nc()
    res = run_bass_kernel_spmd(_CACHE["nc"], in_maps, list(range(NCORES))).results

    S_clash = 0.0
    S_pair = 0.0
    for r in range(NCORES):
        S_clash += float(res[r]["o_clash"].astype(np.float64).sum())
        S_pair += float(res[r]["o_pair"].astype(np.float64).sum())

    band_clash = _band_crossing_clash(AT, BT)
    blk_clash, blk_pair = _host_block_terms(coords, contact_map)

    e_clash = (S_clash - band_clash + blk_clash) / L

    n_pairs = max(int(round(float(contact_map.sum(dtype=np.float64)))), 1)
    e_pair = (S_pair + blk_pair) / n_pairs

    diff = coords.astype(np.float64)[1:] - coords.astype(np.float64)[:-1]
    bond = np.sqrt((diff * diff).sum(axis=1))
    e_bond = float(((bond - IDEAL_BOND) ** 2).mean())

    total = W_BOND * e_bond + W_CLASH * e_clash + W_PAIR * e_pair
    return np.array([total], dtype=np.float32)


# revision 11
# speedup vs baseline: 1.3203x; 1.3203x over previous
"""Energy refinement kernel for Trainium2 (8 NeuronCores, SPMD row-sharded).

Math notes
----------
reference() computes, for L=4096 coords [L,3] and a 0/1 contact_map [L,L]:
  e_bond  = mean((||c[i+1]-c[i]|| - 6)^2)                       (O(L), host)
  d[i,j]  = ||c_i - c_j|| (+1e-8)
  e_clash = sum_{j>=i+3} relu(3.4-d)^2 / L
  e_pair  = sum_{contact & |i-j|>=3} (d-9)^2 / max(n_contacts,1)
  total   = e_bond + 2*e_clash + 0.5*e_pair

Work split: the O(L^2) dense part — the all-pairs distance matrix and the
clash detection/reduction over it — runs on the 8 NeuronCores.  The two
sparse/structured terms run on the host in float64 exactly: e_bond is O(L),
and e_pair touches only the ~167K contact-list pairs (0.2% of the matrix),
where a dense device sweep would waste two full [L,L] passes plus an
[L,L] weight-map DMA per pass.

Device strategy (folded symmetry, row-sharded over 8 cores):
  d2 = A @ B^T in ONE fp16 matmul with a hi/lo split (K=13):
    c = hi + lo (both fp16), |c|^2 = n2hi + n2lo (both fp16); the K slots
    pair up so sum_k A[i,k]B[j,k] = |ci|^2+|cj|^2-2ci.cj to ~0.01 abs
    (the dropped lo*lo cross term is ~1e-2); fp16 streams the PE at 1
    cycle/row vs fp32's 4.
  d  = sqrt(d2 + EPS)  (ACT pass straight from PSUM; the +EPS bias clamps
  fp16-split/accumulation round-off away from negative, replacing a
  separate relu pass; output fp16 so the DVE min/sub runs in 4x mode)
  d is symmetric, so each unordered pair is visited once: a 128-row block a
  only processes the 15-block cyclic column span [128(a+1), 128a+2048) —
  block offsets 1..15 of 32.  Per 128-row tile the device accumulates
  per-partition sums of clash = (min(d,3.4)-3.4)^2 = relu(3.4-d)^2 via a
  DVE min/sub then a column-split square+reduce: the first CLASH_ACT_COLS
  on ACT (Square w/ accum_out), the rest on a DVE scalar_tensor_tensor —
  splitting because ACT (sqrt) and DVE (min/sub) otherwise go idle waiting
  on each other.
  The within-block (offset-0) and offset-16 pairs — the blocks that would
  be double-counted — are ~0.5M pairs and are computed EXACTLY on the host
  in float64 instead.  Per-core column spans ship as data (pre-gathered B
  columns), keeping the SPMD program identical on all cores.
Host finishing (float64): add the exact diag/sep-16 block clash terms,
subtract the block-crossing |i-j|<=2 band pairs (emulated with the
device's own formula so they cancel), divide, add the bond and pair terms.
"""

import numpy as np

L = 4096
NCORES = 8
RPC = L // NCORES          # 512 rows per core
RT = RPC // 128            # 4 row tiles of 128 partitions
SPAN = 15 * 128            # 1920 columns per row tile (block offsets 1..15)
KS = 13                    # fp16-split matmul contraction depth
MIN_DIST = 3.4
TARGET_DIST = 9.0
IDEAL_BOND = 6.0
W_BOND, W_CLASH, W_PAIR = 1.0, 2.0, 0.5
EPS = 0.03                 # d2 clamp bias inside the sqrt activation
# clamp d' = sqrt(d2+EPS) at THR = sqrt(3.4^2+EPS): clash detection then
# happens exactly at d2 = 3.4^2, removing most of the EPS-induced bias
THR = float(np.sqrt(MIN_DIST * MIN_DIST + EPS))
CLASH_ACT_COLS = 512       # clash square+reduce: [0,s) on ACT, [s,SPAN) on DVE


def _build_nc(reps=1):
    import concourse.bass as bass
    import concourse.bacc as bacc
    import concourse.mybir as mybir
    import concourse.tile as tile

    f32 = mybir.dt.float32
    f16 = mybir.dt.float16
    AF = mybir.ActivationFunctionType
    ALU = mybir.AluOpType

    # Bacc (not Bass): its compile() runs move_matmul_waits_to_ldweights,
    # required because walrus allows only one sync wait per Matmult.
    nc = bacc.Bacc(None)
    # ab = [at | bt-span x4] so a single DMA (one wait semaphore) loads all
    # matmul operands — walrus allows only one sync wait per Matmult.
    ab = nc.declare_dram_parameter("ab", [KS, RPC + RT * SPAN], f16, isOutput=False)
    o_clash = nc.declare_dram_parameter("o_clash", [128, 2 * RT], f32, isOutput=True)

    SA = CLASH_ACT_COLS

    with tile.TileContext(nc) as tc:
        with (
            tc.tile_pool(name="const", bufs=1) as constp,
            tc.tile_pool(name="work", bufs=2) as work,
            tc.tile_pool(name="accp", bufs=1) as accp,
            tc.tile_pool(name="psum", bufs=2, space=bass.MemorySpace.PSUM) as psum,
        ):
            ab_sb = constp.tile([KS, RPC + RT * SPAN], f16)
            bias_eps = constp.tile([128, 1], f32)
            nc.gpsimd.memset(bias_eps[:], EPS)
            # split the operand load so row-tile 0's matmuls start after the
            # first chunk instead of waiting for the whole transfer
            nc.sync.dma_start(ab_sb[:, : RPC + 512], ab[:, : RPC + 512])
            nc.sync.dma_start(
                ab_sb[:, RPC + 512 : RPC + SPAN], ab[:, RPC + 512 : RPC + SPAN]
            )
            for it in range(1, RT):
                lo = RPC + it * SPAN
                nc.sync.dma_start(ab_sb[:, lo : lo + SPAN], ab[:, lo : lo + SPAN])

            acc_clash = accp.tile([128, 2 * RT], f32)

            for rep in range(reps):
                for it in range(RT):
                    lhs = ab_sb[:, it * 128 : (it + 1) * 128]
                    rbase = RPC + it * SPAN
                    ps = psum.tile([128, SPAN], f32, tag="d2")
                    off = 0
                    for n in (512, 512, 512, 384):
                        nc.tensor.matmul(
                            ps[:, off : off + n],
                            lhs,
                            ab_sb[:, rbase + off : rbase + off + n],
                            start=True,
                            stop=True,
                        )
                        off += n

                    # d = sqrt(d2 + EPS): EPS clamps matmul round-off, so no
                    # separate relu pass; fp16 output feeds the DVE 4x mode
                    t_d = work.tile([128, SPAN], f16, tag="t_d")
                    nc.scalar.activation(t_d[:], ps[:], AF.Sqrt, bias=bias_eps[:])

                    # clash: m = min(d,3.4)-3.4 (DVE 4x), then m^2 summed —
                    # first SA cols on ACT (Square+accum), rest on DVE stt
                    t_m = work.tile([128, SPAN], f16, tag="t_m")
                    nc.vector.tensor_scalar(
                        t_m[:], t_d[:], THR, THR, ALU.min, ALU.subtract
                    )
                    if SA > 0:
                        t_j0 = work.tile([128, SA], f16, tag="junk_a")
                        nc.scalar.activation(
                            t_j0[:],
                            t_m[:, :SA],
                            AF.Square,
                            accum_out=acc_clash[:, 2 * it : 2 * it + 1],
                        )
                    t_j1 = work.tile([128, SPAN - SA], f16, tag="junk_d")
                    nc.vector.scalar_tensor_tensor(
                        t_j1[:],
                        t_m[:, SA:],
                        1.0,
                        t_m[:, SA:],
                        ALU.mult,
                        ALU.mult,
                        accum_out=acc_clash[:, 2 * it + 1 : 2 * it + 2],
                    )

            nc.sync.dma_start(o_clash[:], acc_clash[:])
    nc.compile()
    return nc


def _split16(x):
    """x (f64/f32) -> (hi, lo) fp16 with hi + lo ~= x."""
    hi = x.astype(np.float16)
    lo = (x.astype(np.float64) - hi.astype(np.float64)).astype(np.float16)
    return hi, lo


def _augmented(coords):
    """AT, BT fp16 [13, L] such that sum_k AT[k,i]*BT[k,j] ~= ||c_i - c_j||^2."""
    c = np.asarray(coords, dtype=np.float64)
    n2 = (c * c).sum(axis=1)
    chi, clo = _split16(c)          # [L,3] each
    n2hi, n2lo = _split16(n2)       # [L]
    one = np.ones(c.shape[0], dtype=np.float16)
    # K slots: 0-2 hi*-2hi, 3-5 hi*-2lo, 6-8 lo*-2hi, 9/10 n2hi/n2lo * 1,
    # 11/12 1 * n2hi/n2lo
    AT = np.stack(
        [chi[:, 0], chi[:, 1], chi[:, 2],
         chi[:, 0], chi[:, 1], chi[:, 2],
         clo[:, 0], clo[:, 1], clo[:, 2],
         n2hi, n2lo, one, one]
    ).astype(np.float16)
    m2hi = (-2.0 * chi.astype(np.float64)).astype(np.float16)
    m2lo = (-2.0 * clo.astype(np.float64)).astype(np.float16)
    BT = np.stack(
        [m2hi[:, 0], m2hi[:, 1], m2hi[:, 2],
         m2lo[:, 0], m2lo[:, 1], m2lo[:, 2],
         m2hi[:, 0], m2hi[:, 1], m2hi[:, 2],
         one, one, n2hi, n2lo]
    ).astype(np.float16)
    return AT, BT


def _host_inputs(coords, contact_map=None):
    AT, BT = _augmented(coords)
    in_maps = []
    for r in range(NCORES):
        parts = [AT[:, r * RPC : (r + 1) * RPC]]
        for it in range(RT):
            a = r * RT + it
            i0 = a * 128
            cols = np.arange(i0 + 128, i0 + 128 + SPAN) % L
            parts.append(BT[:, cols])
        in_maps.append(
            {
                "ab": np.ascontiguousarray(
                    np.concatenate(parts, axis=1), dtype=np.float16
                ),
            }
        )
    return AT, BT, in_maps


def _host_block_clash(coords):
    """Exact f64 clash sum over the diag and sep-16 block pairs
    (the unordered pairs the device span skips), reference mask applied."""
    c = coords.astype(np.float64)
    clash_sum = 0.0
    for a in range(L // 128):
        i0 = a * 128
        # within-block pairs i<j with j-i>=3
        blk = c[i0 : i0 + 128]
        dd = np.sqrt(((blk[:, None, :] - blk[None, :, :]) ** 2).sum(-1)) + 1e-8
        iu, ju = np.triu_indices(128, k=3)
        cl = np.maximum(MIN_DIST - dd[iu, ju], 0.0)
        clash_sum += float((cl * cl).sum())
        # sep-16 block pairs, visited once for a in [0, 16)
        if a < 16:
            j0 = i0 + 2048
            blk2 = c[j0 : j0 + 128]
            d2 = np.sqrt(
                ((blk[:, None, :] - blk2[None, :, :]) ** 2).sum(-1)
            ) + 1e-8
            cl2 = np.maximum(MIN_DIST - d2, 0.0)
            clash_sum += float((cl2 * cl2).sum())
    return clash_sum


def _host_pair(coords, contact_map):
    """e_pair numerator and n_pairs, exactly as the reference: sum over
    directed contacts with |i-j| >= 3 of (||ci-cj|| + 1e-8 - 9)^2."""
    ci, cj = np.nonzero(contact_map > 0.5)
    n_pairs = max(len(ci), 1)
    sel = np.abs(ci - cj) >= 3
    ci, cj = ci[sel], cj[sel]
    c = coords.astype(np.float64)
    d = np.sqrt(((c[ci] - c[cj]) ** 2).sum(axis=1)) + 1e-8
    return float(((d - TARGET_DIST) ** 2).sum()), n_pairs


def _band_crossing_clash(AT, BT):
    """Device-formula clash sums over block-CROSSING |i-j|<=2 pairs
    (the only banded pairs inside the device span), to subtract."""
    band_clash = 0.0
    for s_off in (1, 2):
        i = np.arange(L - s_off)  # non-wrapping pairs only
        i = i[(i % 128) >= 128 - s_off]  # block-crossing only
        j = i + s_off
        s = np.zeros(len(i), dtype=np.float64)
        for m in range(KS):
            s += AT[m, i].astype(np.float64) * BT[m, j].astype(np.float64)
        dh = np.sqrt(np.maximum(s, 0.0) + EPS).astype(np.float16).astype(np.float64)
        cl = np.minimum(dh, THR) - THR
        band_clash += float((cl * cl).sum())
    return band_clash


_CACHE = {}


def kernel(coords, contact_map):
    from concourse.bass_utils import run_bass_kernel_spmd

    coords = np.asarray(coords, dtype=np.float32)
    contact_map = np.asarray(contact_map)
    AT, BT, in_maps = _host_inputs(coords)

    if "nc" not in _CACHE:
        _CACHE["nc"] = _build_nc()
    res = run_bass_kernel_spmd(_CACHE["nc"], in_maps, list(range(NCORES))).results

    S_clash = 0.0
    for r in range(NCORES):
        S_clash += float(res[r]["o_clash"].astype(np.float64).sum())

    band_clash = _band_crossing_clash(AT, BT)
    blk_clash = _host_block_clash(coords)
    e_clash = (S_clash - band_clash + blk_clash) / L

    pair_sum, n_pairs = _host_pair(coords, contact_map)
    e_pair = pair_sum / n_pairs

    diff = coords.astype(np.float64)[1:] - coords.astype(np.float64)[:-1]
    bond = np.sqrt((diff * diff).sum(axis=1))
    e_bond = float(((bond - IDEAL_BOND) ** 2).mean())

    total = W_BOND * e_bond + W_CLASH * e_clash + W_PAIR * e_pair
    return np.array([total], dtype=np.float32)
